# revision 13
# baseline (speedup 1.0000x reference)
"""Trainium2 Bass kernel for MultiHeadSelfAttention with RoPE, value-residual
mix, per-head sigmoid gating, and causal softmax attention.

Sharding (8 NeuronCores): core = batch * 2 + head_group.
Each core handles one batch (of 4) and 8 of the 16 heads:
  - QKV projection column-sharded (its 8 heads), x transposed host-side.
  - Attention for its 8 (head, batch) pairs, fully on-core.
  - Out-projection row-sharded -> partial output; host sums the two
    head-group partials per batch.

Everything heavy runs on TensorE in bf16 with fp32 PSUM accumulation.
Scores stay transposed (k on partitions, q on free) so no on-chip
transposes are needed anywhere; the softmax denominator is computed by
augmenting V with a ones-column; gate/denominator are applied in a single
broadcast multiply on eviction.
"""

from contextlib import ExitStack

import ml_dtypes
import numpy as np

import concourse.bass as bass
import concourse.tile as tile
from concourse import bacc, mybir
from concourse.bass_utils import run_bass_kernel_spmd

B, S, D, H, HD = 4, 2048, 1024, 16, 64
G = 2              # head groups (cores per batch)
HPG = H // G       # heads per group/core
NCORES = B * G
THETA = 10000.0
KT = D // 128      # k-tiles over the D contraction
ST = S // 128      # 128-tiles over sequence
QC = S // 512      # 512-wide q-chunks
BF16 = mybir.dt.bfloat16
F32 = mybir.dt.float32
NEG = -30000.0

_CACHE: dict = {}


def _build_program():
    nc = bacc.Bacc("TRN2", target_bir_lowering=False, debug=False,
                   num_devices=NCORES)

    xT = nc.dram_tensor("xT", [D, S], BF16, kind="ExternalInput").ap()
    wqk = nc.dram_tensor("wqk", [D, 2 * HPG * HD], BF16, kind="ExternalInput").ap()
    wv = nc.dram_tensor("wv", [D, HPG * 65], BF16, kind="ExternalInput").ap()
    v1p = nc.dram_tensor("v1p", [HPG, S, 65], BF16, kind="ExternalInput").ap()
    wout = nc.dram_tensor("wout", [HPG * HD, D], BF16, kind="ExternalInput").ap()
    wg = nc.dram_tensor("wg", [D, HPG], BF16, kind="ExternalInput").ap()
    cs = nc.dram_tensor("cs", [128, 2 * S], F32, kind="ExternalInput").ap()
    outp = nc.dram_tensor("outp", [S, D], F32, kind="ExternalOutput").ap()
    vout = nc.dram_tensor("vout", [HPG, S, HD], BF16, kind="ExternalOutput").ap()

    gate_d = nc.dram_tensor("gate_bounce", [HPG, S], F32).ap()

    with tile.TileContext(nc) as tc, ExitStack() as ctx:
        res = ctx.enter_context(tc.tile_pool(name="resident", bufs=1))
        sp_pool = ctx.enter_context(tc.tile_pool(name="sp", bufs=3))
        ex_pool = ctx.enter_context(tc.tile_pool(name="expt", bufs=4))
        oe_pool = ctx.enter_context(tc.tile_pool(name="oev", bufs=2))
        cf_pool = ctx.enter_context(tc.tile_pool(name="coef", bufs=2))
        ps_mm = ctx.enter_context(tc.tile_pool(name="ps_mm", bufs=4, space="PSUM"))
        ps_rope = ctx.enter_context(tc.tile_pool(name="ps_rope", bufs=2, space="PSUM"))
        ps_pv = ctx.enter_context(tc.tile_pool(name="ps_pv", bufs=2, space="PSUM"))

        # ---- resident SBUF tensors ----
        xts = res.tile([128, KT, S], BF16)
        wqks = res.tile([128, KT, 2 * HPG * HD], BF16)
        wvs = res.tile([128, KT, HPG * 65], BF16)
        wouts = res.tile([128, HPG * HD // 128, D], BF16)
        wgs = res.tile([128, KT, HPG], BF16)
        css = res.tile([128, 2 * S], F32)
        vm = res.tile([128, ST, HPG, 65], BF16)
        qrot = res.tile([128, HPG * HD // 128, S], BF16)
        krot = res.tile([128, HPG * HD // 128, S], BF16)
        af = res.tile([128, HPG * HD // 128, S], BF16)

        # ---- input DMAs ----
        for kt in range(KT):
            nc.sync.dma_start(out=xts[:, kt, :], in_=xT[kt * 128:(kt + 1) * 128, :])
            nc.sync.dma_start(out=wqks[:, kt, :], in_=wqk[kt * 128:(kt + 1) * 128, :])
            nc.sync.dma_start(out=wvs[:, kt, :], in_=wv[kt * 128:(kt + 1) * 128, :])
        for r in range(HPG * HD // 128):
            nc.sync.dma_start(out=wouts[:, r, :], in_=wout[r * 128:(r + 1) * 128, :])
        nc.sync.dma_start(out=wgs, in_=wg.rearrange("(kt p) h -> p kt h", p=128))
        nc.sync.dma_start(out=css, in_=cs)

        # ---- gate: sigmoid(Wg.T @ xT) [HPG, S] -> DRAM bounce (before exps) ----
        for tq in range(QC):
            ps = ps_mm.tile([HPG, 512], F32, tag="mm")
            for kt in range(KT):
                nc.tensor.matmul(ps, wgs[:, kt, :],
                                 xts[:, kt, tq * 512:(tq + 1) * 512],
                                 start=(kt == 0), stop=(kt == KT - 1))
            sg = sp_pool.tile([HPG, 512], F32, tag="sg")
            nc.scalar.activation(sg, ps, mybir.ActivationFunctionType.Sigmoid)
            nc.sync.dma_start(out=gate_d[:, tq * 512:(tq + 1) * 512], in_=sg)

        # ---- Q/K projection + RoPE (rotate-half layout, tables host-built) ----
        # jt 0..3 -> qrot tiles, jt 4..7 -> krot tiles; each 128-row tile holds
        # two heads (rows h*64 + [x1(32) | x2(32)]).
        for jt in range(2 * HPG * HD // 128):
            dst = qrot if jt < 4 else krot
            dj = jt % 4
            for tq in range(QC):
                t0 = tq * 512
                ps = ps_mm.tile([128, 512], F32, tag="mm")
                for kt in range(KT):
                    nc.tensor.matmul(ps, wqks[:, kt, jt * 128:(jt + 1) * 128],
                                     xts[:, kt, t0:t0 + 512],
                                     start=(kt == 0), stop=(kt == KT - 1))
                cp = ps_rope.tile([128, 512], F32, tag="rope")
                sp = sp_pool.tile([128, 512], F32)
                nc.vector.tensor_mul(cp, ps, css[:, t0:t0 + 512])
                nc.vector.tensor_mul(sp, ps, css[:, S + t0:S + t0 + 512])
                for hh in range(2):
                    b0 = hh * 64
                    d = dst[:, dj, t0:t0 + 512]
                    # out1 = x1*cos - x2*sin ; out2 = x1*sin + x2*cos
                    nc.vector.tensor_sub(d[b0:b0 + 32], cp[b0:b0 + 32],
                                         sp[b0 + 32:b0 + 64])
                    nc.vector.tensor_add(d[b0 + 32:b0 + 64], sp[b0:b0 + 32],
                                         cp[b0 + 32:b0 + 64])

        # ---- V projection + residual mix (+ ones column for row sums) ----
        for tk in range(ST):
            for nh in range(2):
                c0 = nh * 4 * 65
                v1t = sp_pool.tile([128, 260], BF16, tag="v1t")
                nc.sync.dma_start(
                    out=v1t.rearrange("q (p d) -> q p d", p=4),
                    in_=v1p[nh * 4:(nh + 1) * 4, tk * 128:(tk + 1) * 128, :]
                    .rearrange("p q d -> q p d"))
                ps = ps_mm.tile([128, 260], F32, tag="mm")
                for kt in range(KT):
                    nc.tensor.matmul(ps, xts[:, kt, tk * 128:(tk + 1) * 128],
                                     wvs[:, kt, c0:c0 + 260],
                                     start=(kt == 0), stop=(kt == KT - 1))
                nc.vector.tensor_add(vm[:, tk, nh * 4:(nh + 1) * 4, :], ps, v1t)
        for p in range(HPG):
            nc.sync.dma_start(
                out=vout[p].rearrange("(tk q) d -> q tk d", q=128),
                in_=vm[:, :, p, 0:HD])

        # ---- causal attention, scores transposed [k, q] ----
        for qt in range(QC):
            q0 = qt * 512
            for pair in range(HPG):
                jt = pair // 2
                b0 = (pair % 2) * 64
                pv = ps_pv.tile([65, 512], F32, tag="pv")
                kt_hi = qt * 4 + 3
                for kt in range(kt_hi + 1):
                    c = max(0, kt * 128 - q0)
                    n = 512 - c
                    sc = ps_mm.tile([128, 512], F32, tag="mm")
                    nc.tensor.matmul(sc[:, 0:n],
                                     krot[b0:b0 + 64, jt, kt * 128:(kt + 1) * 128],
                                     qrot[b0:b0 + 64, jt, q0 + c:q0 + 512],
                                     start=True, stop=True)
                    et = ex_pool.tile([128, 512], BF16)
                    nc.scalar.activation(et[:, 0:n], sc[:, 0:n],
                                         mybir.ActivationFunctionType.Exp)
                    if kt >= qt * 4:
                        # diagonal block: zero strictly-lower (k > q) entries
                        nc.gpsimd.affine_select(
                            out=et[:, 0:128], in_=et[:, 0:128],
                            compare_op=mybir.AluOpType.is_ge, fill=0.0,
                            base=0, pattern=[[1, 128]], channel_multiplier=-1)
                    nc.tensor.matmul(pv[:, c:512], vm[:, kt, pair, :], et[:, 0:n],
                                     start=(kt == 0), stop=(kt == kt_hi))
                # coef = gate / Z ; broadcast along d; evict attn^T * coef
                gcf = cf_pool.tile([1, 512], F32, tag="g")
                nc.sync.dma_start(out=gcf, in_=gate_d[pair:pair + 1, q0:q0 + 512])
                rzt = cf_pool.tile([1, 512], F32, tag="rz")
                nc.vector.reciprocal(rzt, pv[64:65, :])
                cft = cf_pool.tile([1, 512], F32, tag="cf")
                nc.vector.tensor_mul(cft, rzt, gcf)
                cfb = cf_pool.tile([64, 512], F32, tag="cfb")
                nc.gpsimd.partition_broadcast(cfb, cft)
                nc.vector.tensor_mul(af[b0:b0 + 64, jt, q0:q0 + 512],
                                     pv[0:64, :], cfb)
            # ---- out projection for this q-range (all heads now final) ----
            for tt in range(qt * 4, qt * 4 + 4):
                for jc in range(2):
                    ps = ps_mm.tile([128, 512], F32, tag="mm")
                    for k2 in range(HPG * HD // 128):
                        nc.tensor.matmul(ps, af[:, k2, tt * 128:(tt + 1) * 128],
                                         wouts[:, k2, jc * 512:(jc + 1) * 512],
                                         start=(k2 == 0), stop=(k2 == 3))
                    ot = oe_pool.tile([128, 512], F32, tag="oev")
                    nc.any.tensor_copy(ot, ps)
                    nc.sync.dma_start(
                        out=outp[tt * 128:(tt + 1) * 128, jc * 512:(jc + 1) * 512],
                        in_=ot)

    nc.compile()
    return nc


def _get_compiled():
    if "nc" not in _CACHE:
        _CACHE["nc"] = _build_program()
    return _CACHE["nc"]


def _prep_inputs(x, v1, token_positions, Wqkv, Wout, Wgate, alpha1, alpha2, scale):
    bf = ml_dtypes.bfloat16
    f32 = np.float32
    x = np.asarray(x, f32)
    v1 = np.asarray(v1, f32)
    pos = np.asarray(token_positions)
    Wqkv = np.asarray(Wqkv, f32)
    Wout = np.asarray(Wout, f32)
    Wgate = np.asarray(Wgate, f32)
    a1 = f32(alpha1)
    a2 = f32(alpha2)
    sc = f32(scale)
    rden = f32(1.0) / np.sqrt(a1 * a1 + a2 * a2 + f32(1e-8))
    c1 = sc * a1 * rden
    c2 = sc * a2 * rden

    perm = np.concatenate([np.arange(0, HD, 2), np.arange(1, HD, 2)])
    Wq = Wqkv[:, 0:D]
    Wk = Wqkv[:, D:2 * D]
    Wv = Wqkv[:, 2 * D:3 * D]

    inv_freq = (1.0 / (THETA ** (np.arange(0, HD, 2, dtype=f32) / HD))).astype(f32)

    # per-head-group weights
    group = []
    for g in range(G):
        wqk_g = np.empty((D, 2 * HPG * HD), f32)
        wv_g = np.zeros((D, HPG * 65), f32)
        for j in range(HPG):
            h = g * HPG + j
            wqk_g[:, j * HD:(j + 1) * HD] = Wq[:, h * HD + perm] * f32(1.0 / np.sqrt(HD))
            wqk_g[:, HPG * HD + j * HD:HPG * HD + (j + 1) * HD] = Wk[:, h * HD + perm]
            wv_g[:, j * 65:j * 65 + HD] = Wv[:, h * HD:(h + 1) * HD] * c1
        wout_g = Wout[g * HPG * HD:(g + 1) * HPG * HD, :]
        wg_g = Wgate[:, g * HPG:(g + 1) * HPG]
        group.append((wqk_g.astype(bf), wv_g.astype(bf),
                      wout_g.astype(bf), wg_g.astype(bf)))

    in_maps = []
    for b in range(B):
        xT_b = np.ascontiguousarray(x[b].T).astype(bf)
        ang = pos[b].astype(f32)[None, :] * inv_freq[:, None]  # [32, S] fp32
        cos = np.cos(ang).astype(f32)
        sin = np.sin(ang).astype(f32)
        cs_b = np.concatenate([np.tile(cos, (4, 1)), np.tile(sin, (4, 1))],
                              axis=1).astype(f32)  # [128, 2S]
        for g in range(G):
            wqk_g, wv_g, wout_g, wg_g = group[g]
            v1p = np.ones((HPG, S, 65), f32)
            for j in range(HPG):
                v1p[j, :, 0:HD] = v1[(g * HPG + j) * B + b] * c2
            in_maps.append({
                "xT": xT_b, "wqk": wqk_g, "wv": wv_g,
                "v1p": v1p.astype(bf), "wout": wout_g, "wg": wg_g,
                "cs": cs_b,
            })
    return in_maps


def _assemble(results):
    attn_out = np.empty((B, S, D), np.float32)
    V = np.empty((H * B, S, HD), np.float32)
    for b in range(B):
        attn_out[b] = results[b * G + 0]["outp"] + results[b * G + 1]["outp"]
        for g in range(G):
            vo = results[b * G + g]["vout"].astype(np.float32)
            for j in range(HPG):
                V[(g * HPG + j) * B + b] = vo[j]
    return attn_out, V


def run_kernel(inputs, trace=False):
    nc = _get_compiled()
    in_maps = _prep_inputs(**inputs)
    res = run_bass_kernel_spmd(nc, in_maps, list(range(NCORES)), trace=trace)
    return _assemble(res.results), res.exec_time_ns


def kernel(**inputs):
    out, _ = run_kernel(inputs, trace=False)
    return out
